# revision 25
# baseline (speedup 1.0000x reference)
"""Trainium2 Bass kernel for nn_BinarizedLinear:
    out = sign(input_b @ sign(weight).T)
with input_b (8192, 2048) and weight (2048, 2048), entries all +/-1.0 fp32.

Since weight entries are +/-1, sign(weight) == weight; the linear output is a
sum of 2048 +/-1 terms, i.e. an even integer in [-2048, 2048], so
sign(v) == clamp(v, -1, 1) exactly, and bf16/fp8 operands are bit-exact
(+/-1 is representable; PSUM accumulates in fp32).

Strategy: data-parallel across 8 NeuronCores - each core gets 1024 rows of
input_b, the full weight replicated.  Per core both the DMA pool (32MB at
~23.4GB/s x 16 engines ~= 97us of engine time) and the TensorEngine
(54.6us fp8-DR matmul + ~42us of operand transposes) sit near 100us, so the
kernel aims to keep the PE 100% busy on useful work from the first slab on:
  - x/W fp32 slabs DMA in as full [128, 2048] rows (8KB contiguous lines),
  - DVE casts fp32 -> bf16 (dedicated engine, never queues behind evictions),
  - PE transposes 128x128 sub-tiles (107ns each), 8 packed per PSUM bank so
    each ACT eviction moves 1024 columns, casting bf16 -> fp8e4 in the copy,
  - fp8 matmuls with perf_mode=DoubleRow (2 k-tiles/pass, 0.5 cyc/row)
    accumulate k=2048 into PSUM fp32 - exact since products are +/-1,
  - sign() fuses into the PSUM->SBUF eviction as one ACT Sign activation
    (exact at 0), results DMA out on the GpSimd SWDGE queue.
Engine assignment keeps every in-order queue a single dependency stream:
DVE only casts, ACT evicts then signs (each sign's matmul has retired by
the time ACT reaches it), so no queue ever head-of-line blocks the PE feed.
A short fp8-DR dummy warmup (213ns each, vs 426ns for bf16) flips the HAM
clock gate to full rate during the DMA-only head without burning real PE
time; small bursts continue between early slabs because HAM ignores
transpose-mode activity.  W slabs are produced W-heavy early so o-block 0
closes fast and matmuls interleave with the remaining production.
"""

import numpy as np

BATCH, IN_LEN, OUT_LEN = 8192, 2048, 2048
N_CORES = 8
SHARD = BATCH // N_CORES  # 1024
P = 128

_cache = {}


def build_kernel(shard=SHARD, in_len=IN_LEN, out_len=OUT_LEN,
                 warm0=8, warm_group=2, warm_units=5, mm_lag_flush=3):
    import concourse.mybir as mybir
    import concourse.tile as tile
    from concourse import bacc
    from concourse.masks import make_identity

    f32 = mybir.dt.float32
    bf16 = mybir.dt.bfloat16
    fp8 = mybir.dt.float8e4

    KT = in_len // P          # 16 k-tiles (contraction)
    BT = shard // P           # 8 b-tiles per core
    OB = out_len // 512       # 4 o-blocks of 512
    WS = out_len // P         # 16 W slabs of 128 rows
    KP = 8                    # transposes packed per PSUM bank eviction

    nc = bacc.Bacc(None, target_bir_lowering=False)
    x = nc.dram_tensor("x", [shard, in_len], f32, kind="ExternalInput")
    w = nc.dram_tensor("w", [out_len, in_len], f32, kind="ExternalInput")
    out = nc.dram_tensor("out", [shard, out_len], f32, kind="ExternalOutput")
    scratch = nc.dram_tensor("scratch", [1, 1], f32, kind="ExternalOutput")

    with tile.TileContext(nc) as tc:
        with (
            tc.tile_pool(name="const", bufs=1) as const_pool,
            tc.tile_pool(name="fstage", bufs=8) as fstage_pool,
            tc.tile_pool(name="bstage", bufs=8) as bstage_pool,
            tc.tile_pool(name="xt", bufs=BT) as xt_pool,
            tc.tile_pool(name="wt", bufs=OB) as wt_pool,
            tc.tile_pool(name="outs", bufs=10) as out_pool,
            tc.tile_pool(name="tpsum", bufs=3, space="PSUM") as tpsum_pool,
            tc.tile_pool(name="mpsum", bufs=4, space="PSUM") as mpsum_pool,
            tc.tile_pool(name="wpsum", bufs=1, space="PSUM") as wpsum_pool,
        ):
            # HAM warmup: PE is otherwise idle during the DMA-bound head;
            # dummy fp8 DR matmuls keep the PE clock gate at full rate.
            warm_src = const_pool.tile([P, 2, 512], fp8, name="warm_src")
            nc.gpsimd.memset(warm_src[:], 1.0)
            warm_psum = wpsum_pool.tile([P, 512], f32, name="warm_psum")

            def warm_burst(n):
                for i in range(n):
                    nc.tensor.matmul(
                        warm_psum[:], warm_src[:, :, :P], warm_src[:],
                        start=(i == 0), stop=(i == n - 1),
                        perf_mode=mybir.MatmulPerfMode.DoubleRow,
                    )

            warm_burst(warm0)
            warm_out = const_pool.tile([1, 1], f32, name="warm_out")
            nc.vector.tensor_copy(out=warm_out[:], in_=warm_psum[:1, :1])
            nc.gpsimd.dma_start(out=scratch[:], in_=warm_out[:])

            ident = const_pool.tile([P, P], bf16)
            make_identity(nc, ident)

            # resident fp8 operands
            # xt[bt][p, k, b] = x[bt*128 + b, k*128 + p]
            # wt[ob][p, k, o] = w[ob*512 + o, k*128 + p]
            xt = {bt: xt_pool.tile([P, KT, P], fp8, tag="xt", name=f"xt{bt}")
                  for bt in range(BT)}
            wt = {ob: wt_pool.tile([P, KT, 512], fp8, tag="wt", name=f"wt{ob}")
                  for ob in range(OB)}

            def produce(dram, row0, dest_fn):
                """DMA a full [128, in_len] fp32 slab, cast to bf16 on DVE
                in halves (finer transpose pipelining), PE-transpose 128x128
                sub-tiles packed KP per PSUM bank, then evict each bank
                (ACT, fused fp8 cast) via dest_fn."""
                fs = fstage_pool.tile([P, in_len], f32, tag="fs")
                nc.sync.dma_start(out=fs[:], in_=dram[row0:row0 + P, :])
                bs = bstage_pool.tile([P, in_len], bf16, tag="bs")
                half = in_len // 2
                nc.vector.tensor_copy(out=bs[:, :half], in_=fs[:, :half])
                nc.vector.tensor_copy(out=bs[:, half:], in_=fs[:, half:])
                for k0 in range(0, KT, KP):
                    tp = tpsum_pool.tile([P, KP * P], bf16, tag="tp")
                    for q in range(KP):
                        col = (k0 + q) * P
                        nc.tensor.transpose(
                            tp[:, q * P:(q + 1) * P],
                            bs[:, col:col + P],
                            ident[:],
                        )
                    dest_fn(k0, tp)

            def emit_x(bt):
                def dest(k0, tp):
                    nc.scalar.copy(
                        out=xt[bt][:, k0:k0 + KP, :],
                        in_=tp[:].rearrange("p (k b) -> p k b", k=KP),
                    )
                produce(x, bt * P, dest)

            def emit_w(s):
                ob, j = s // 4, s % 4

                def dest(k0, tp):
                    nc.scalar.copy(
                        out=wt[ob][:, k0:k0 + KP, j * P:(j + 1) * P],
                        in_=tp[:].rearrange("p (k b) -> p k b", k=KP),
                    )
                produce(w, s * P, dest)

            def emit_mm(ob, bt, out_eng=None):
                psum = mpsum_pool.tile([P, 512], f32)
                for q in range(KT // 2):
                    nc.tensor.matmul(
                        psum[:],
                        xt[bt][:, 2 * q:2 * q + 2, :],
                        wt[ob][:, 2 * q:2 * q + 2, :],
                        start=(q == 0),
                        stop=(q == KT // 2 - 1),
                        perf_mode=mybir.MatmulPerfMode.DoubleRow,
                    )
                ot = out_pool.tile([P, 512], f32)
                # sign() on ACT so the DVE queue stays pure casts (a sign
                # waiting on its matmul would head-of-line block the next
                # slab's cast and starve the PE of transposes)
                nc.scalar.sign(out=ot[:], in_=psum[:])
                (out_eng or nc.gpsimd).dma_start(
                    out=out[bt * P:(bt + 1) * P, ob * 512:(ob + 1) * 512],
                    in_=ot[:],
                )

            # Production order: wt[0] closes after 4 units and x0/x1 follow
            # immediately so matmuls start early; x2-x4 interleave to keep
            # the matmul supply >= ~0.6/unit (PE never starves), the W tail
            # closes ob1..ob3, and the final x slabs each unlock one ready
            # matmul per o-block for a short drain.
            production = ([("w", s) for s in range(4)]
                          + [("x", 0), ("x", 1)]
                          + [("w", s) for s in range(4, 8)]
                          + [("x", 2), ("w", 8), ("x", 3), ("w", 9),
                             ("x", 4), ("w", 10), ("w", 11)]
                          + [("w", s) for s in range(12, WS)]
                          + [("x", 5), ("x", 6), ("x", 7)])

            x_done, w_done = set(), set()
            mm_todo = [(ob, bt) for ob in range(OB) for bt in range(BT)]

            def flush_mms(limit, xset, wset, out_eng=None):
                n = 0
                for item in list(mm_todo):
                    ob, bt = item
                    if ob in wset and bt in xset and n < limit:
                        emit_mm(ob, bt, out_eng)
                        mm_todo.remove(item)
                        n += 1

            # flush with one-production-lag availability so a matmul block
            # never waits on the eviction of the slab emitted right before it
            prev_x, prev_w = set(), set()
            for u, item in enumerate(production):
                if 0 < u <= warm_units:
                    warm_burst(warm_group)
                if item[0] == "x":
                    emit_x(item[1])
                    x_done.add(item[1])
                else:
                    emit_w(item[1])
                    if item[1] % 4 == 3:
                        w_done.add(item[1] // 4)
                flush_mms(mm_lag_flush, prev_x, prev_w)
                prev_x, prev_w = set(x_done), set(w_done)
            flush_mms(len(mm_todo), x_done, w_done, out_eng=nc.sync)

    nc.finalize()
    return nc


def _get_nc():
    if "nc" not in _cache:
        _cache["nc"] = build_kernel()
    return _cache["nc"]


def run_sharded(input_b, weight, trace=False):
    """Run the SPMD kernel; returns (output, BassKernelResults)."""
    from concourse.bass_utils import run_bass_kernel_spmd

    nc = _get_nc()
    input_b = np.ascontiguousarray(input_b, dtype=np.float32)
    weight = np.ascontiguousarray(weight, dtype=np.float32)
    in_maps = [
        {"x": input_b[c * SHARD:(c + 1) * SHARD], "w": weight}
        for c in range(N_CORES)
    ]
    res = run_bass_kernel_spmd(nc, in_maps, list(range(N_CORES)), trace=trace)
    out = np.concatenate([res.results[c]["out"] for c in range(N_CORES)], axis=0)
    return out, res


def kernel(input_b, weight):
    out, _ = run_sharded(input_b, weight, trace=False)
    return out


# revision 26
# speedup vs baseline: 1.1516x; 1.1516x over previous
"""Trainium2 Bass kernel for nn_BinarizedLinear:
    out = sign(input_b @ sign(weight).T)
with input_b (8192, 2048) and weight (2048, 2048), entries all +/-1.0 fp32.

Since weight entries are +/-1, sign(weight) == weight; the linear output is a
sum of 2048 +/-1 terms, i.e. an even integer in [-2048, 2048], so
sign(v) == clamp(v, -1, 1) exactly, and bf16/fp8 operands are bit-exact
(+/-1 is representable; PSUM accumulates in fp32).

Strategy: data-parallel across 8 NeuronCores - each core gets 1024 rows of
input_b, the full weight replicated.  Per core both the DMA pool (32MB at
~23.4GB/s x 16 engines ~= 97us of engine time) and the TensorEngine
(54.6us fp8-DR matmul + ~42us of operand transposes) sit near 100us, so the
kernel aims to keep the PE 100% busy on useful work from the first slab on:
  - x/W fp32 slabs DMA in as full [128, 2048] rows (8KB contiguous lines),
  - DVE casts fp32 -> bf16 (dedicated engine, never queues behind evictions),
  - PE transposes 128x128 sub-tiles (107ns each), 8 packed per PSUM bank so
    each ACT eviction moves 1024 columns, casting bf16 -> fp8e4 in the copy,
  - fp8 matmuls with perf_mode=DoubleRow (2 k-tiles/pass, 0.5 cyc/row)
    accumulate k=2048 into PSUM fp32 - exact since products are +/-1,
  - sign() fuses into the PSUM->SBUF eviction as one ACT Sign activation
    (exact at 0), results DMA out on the GpSimd SWDGE queue.
Engine assignment keeps every in-order queue a single dependency stream:
DVE only casts, ACT evicts then signs (each sign's matmul has retired by
the time ACT reaches it), so no queue ever head-of-line blocks the PE feed.
A short fp8-DR dummy warmup (213ns each, vs 426ns for bf16) flips the HAM
clock gate to full rate during the DMA-only head without burning real PE
time; small bursts continue between early slabs because HAM ignores
transpose-mode activity.  W slabs are produced W-heavy early so o-block 0
closes fast and matmuls interleave with the remaining production.
"""

import numpy as np

BATCH, IN_LEN, OUT_LEN = 8192, 2048, 2048
N_CORES = 8
SHARD = BATCH // N_CORES  # 1024
P = 128

_cache = {}


def build_kernel(shard=SHARD, in_len=IN_LEN, out_len=OUT_LEN,
                 warm0=8, warm_group=2, warm_units=5, mm_lag_flush=3):
    import concourse.mybir as mybir
    import concourse.tile as tile
    from concourse import bacc
    from concourse.masks import make_identity

    f32 = mybir.dt.float32
    bf16 = mybir.dt.bfloat16
    fp8 = mybir.dt.float8e4

    KT = in_len // P          # 16 k-tiles (contraction)
    BT = shard // P           # 8 b-tiles per core
    OB = out_len // 512       # 4 o-blocks of 512
    WS = out_len // P         # 16 W slabs of 128 rows
    KP = 8                    # transposes packed per PSUM bank eviction

    nc = bacc.Bacc(None, target_bir_lowering=False)
    x = nc.dram_tensor("x", [shard, in_len], f32, kind="ExternalInput")
    w = nc.dram_tensor("w", [out_len, in_len], f32, kind="ExternalInput")
    out = nc.dram_tensor("out", [shard, out_len], f32, kind="ExternalOutput")
    scratch = nc.dram_tensor("scratch", [1, 1], f32, kind="ExternalOutput")

    with tile.TileContext(nc) as tc:
        with (
            tc.tile_pool(name="const", bufs=1) as const_pool,
            tc.tile_pool(name="fstage", bufs=6) as fstage_pool,
            tc.tile_pool(name="bstage", bufs=6) as bstage_pool,
            tc.tile_pool(name="xt", bufs=BT) as xt_pool,
            tc.tile_pool(name="wt", bufs=OB) as wt_pool,
            tc.tile_pool(name="outs", bufs=10) as out_pool,
            tc.tile_pool(name="tpsum", bufs=3, space="PSUM") as tpsum_pool,
            tc.tile_pool(name="mpsum", bufs=4, space="PSUM") as mpsum_pool,
            tc.tile_pool(name="wpsum", bufs=1, space="PSUM") as wpsum_pool,
        ):
            # HAM warmup: PE is otherwise idle during the DMA-bound head;
            # dummy fp8 DR matmuls keep the PE clock gate at full rate.
            warm_src = const_pool.tile([P, 2, 512], fp8, name="warm_src")
            nc.gpsimd.memset(warm_src[:], 1.0)
            warm_psum = wpsum_pool.tile([P, 512], f32, name="warm_psum")

            def warm_burst(n):
                for i in range(n):
                    nc.tensor.matmul(
                        warm_psum[:], warm_src[:, :, :P], warm_src[:],
                        start=(i == 0), stop=(i == n - 1),
                        perf_mode=mybir.MatmulPerfMode.DoubleRow,
                    )

            warm_burst(warm0)
            warm_out = const_pool.tile([1, 1], f32, name="warm_out")
            nc.vector.tensor_copy(out=warm_out[:], in_=warm_psum[:1, :1])
            nc.gpsimd.dma_start(out=scratch[:], in_=warm_out[:])

            ident = const_pool.tile([P, P], bf16)
            make_identity(nc, ident)

            # resident fp8 operands
            # xt[bt][p, k, b] = x[bt*128 + b, k*128 + p]
            # wt[ob][p, k, o] = w[ob*512 + o, k*128 + p]
            xt = {bt: xt_pool.tile([P, KT, P], fp8, tag="xt", name=f"xt{bt}")
                  for bt in range(BT)}
            wt = {ob: wt_pool.tile([P, KT, 512], fp8, tag="wt", name=f"wt{ob}")
                  for ob in range(OB)}

            def produce(dram, row0, dest_fn):
                """DMA a full [128, in_len] fp32 slab, cast to bf16 on DVE
                in halves (finer transpose pipelining), PE-transpose 128x128
                sub-tiles packed KP per PSUM bank, then evict each bank
                (ACT, fused fp8 cast) via dest_fn."""
                fs = fstage_pool.tile([P, in_len], f32, tag="fs")
                nc.sync.dma_start(out=fs[:], in_=dram[row0:row0 + P, :])
                bs = bstage_pool.tile([P, in_len], bf16, tag="bs")
                half = in_len // 2
                nc.vector.tensor_copy(out=bs[:, :half], in_=fs[:, :half])
                nc.vector.tensor_copy(out=bs[:, half:], in_=fs[:, half:])
                for k0 in range(0, KT, KP):
                    tp = tpsum_pool.tile([P, KP * P], bf16, tag="tp")
                    for q in range(KP):
                        col = (k0 + q) * P
                        nc.tensor.transpose(
                            tp[:, q * P:(q + 1) * P],
                            bs[:, col:col + P],
                            ident[:],
                        )
                    dest_fn(k0, tp)

            def emit_x(bt):
                def dest(k0, tp):
                    nc.scalar.copy(
                        out=xt[bt][:, k0:k0 + KP, :],
                        in_=tp[:].rearrange("p (k b) -> p k b", k=KP),
                    )
                produce(x, bt * P, dest)

            def emit_w(s):
                ob, j = s // 4, s % 4

                def dest(k0, tp):
                    nc.scalar.copy(
                        out=wt[ob][:, k0:k0 + KP, j * P:(j + 1) * P],
                        in_=tp[:].rearrange("p (k b) -> p k b", k=KP),
                    )
                produce(w, s * P, dest)

            def emit_mm(ob, bt, out_eng=None):
                psum = mpsum_pool.tile([P, 512], f32)
                for q in range(KT // 2):
                    nc.tensor.matmul(
                        psum[:],
                        xt[bt][:, 2 * q:2 * q + 2, :],
                        wt[ob][:, 2 * q:2 * q + 2, :],
                        start=(q == 0),
                        stop=(q == KT // 2 - 1),
                        perf_mode=mybir.MatmulPerfMode.DoubleRow,
                    )
                ot = out_pool.tile([P, 512], f32)
                # sign() on ACT so the DVE queue stays pure casts (a sign
                # waiting on its matmul would head-of-line block the next
                # slab's cast and starve the PE of transposes)
                nc.scalar.sign(out=ot[:], in_=psum[:])
                (out_eng or nc.gpsimd).dma_start(
                    out=out[bt * P:(bt + 1) * P, ob * 512:(ob + 1) * 512],
                    in_=ot[:],
                )

            # Production order: wt[0] closes after 4 units and x0/x1 follow
            # immediately so matmuls start early; x2-x4 interleave to keep
            # the matmul supply >= ~0.6/unit (PE never starves), the W tail
            # closes ob1..ob3, and the final x slabs each unlock one ready
            # matmul per o-block for a short drain.
            production = ([("w", s) for s in range(4)]
                          + [("x", 0), ("x", 1)]
                          + [("w", s) for s in range(4, 8)]
                          + [("x", 2), ("w", 8), ("x", 3), ("w", 9),
                             ("x", 4), ("w", 10), ("w", 11)]
                          + [("w", s) for s in range(12, WS)]
                          + [("x", 5), ("x", 6), ("x", 7)])

            x_done, w_done = set(), set()
            mm_todo = [(ob, bt) for ob in range(OB) for bt in range(BT)]

            def flush_mms(limit, xset, wset, out_eng=None):
                n = 0
                for item in list(mm_todo):
                    ob, bt = item
                    if ob in wset and bt in xset and n < limit:
                        emit_mm(ob, bt, out_eng)
                        mm_todo.remove(item)
                        n += 1

            # flush with one-production-lag availability so a matmul block
            # never waits on the eviction of the slab emitted right before it
            prev_x, prev_w = set(), set()
            for u, item in enumerate(production):
                if 0 < u <= warm_units:
                    warm_burst(warm_group)
                if item[0] == "x":
                    emit_x(item[1])
                    x_done.add(item[1])
                else:
                    emit_w(item[1])
                    if item[1] % 4 == 3:
                        w_done.add(item[1] // 4)
                flush_mms(mm_lag_flush, prev_x, prev_w)
                prev_x, prev_w = set(x_done), set(w_done)
            flush_mms(len(mm_todo), x_done, w_done, out_eng=nc.sync)

    nc.finalize()
    return nc


def _get_nc():
    if "nc" not in _cache:
        _cache["nc"] = build_kernel()
    return _cache["nc"]


def run_sharded(input_b, weight, trace=False):
    """Run the SPMD kernel; returns (output, BassKernelResults)."""
    from concourse.bass_utils import run_bass_kernel_spmd

    nc = _get_nc()
    input_b = np.ascontiguousarray(input_b, dtype=np.float32)
    weight = np.ascontiguousarray(weight, dtype=np.float32)
    in_maps = [
        {"x": input_b[c * SHARD:(c + 1) * SHARD], "w": weight}
        for c in range(N_CORES)
    ]
    res = run_bass_kernel_spmd(nc, in_maps, list(range(N_CORES)), trace=trace)
    out = np.concatenate([res.results[c]["out"] for c in range(N_CORES)], axis=0)
    return out, res


def kernel(input_b, weight):
    out, _ = run_sharded(input_b, weight, trace=False)
    return out
